# revision 16
# baseline (speedup 1.0000x reference)
"""AttentionLSTM kernel for 8 Trainium2 NeuronCores.

Data-parallel over batch: B=64 -> 8 rows per core. Each core runs the
embedding gather, the x@Wi projection, the full T=512 LSTM scan on its
batch shard, and the attention tail. Weights are replicated.

Self-contained: hardcodes all shapes; imports only installed packages.
"""

import os
import sys
import types
import contextlib

import numpy as np

import concourse.bass as bass
import concourse.mybir as mybir
import concourse.tile as tile
from concourse.bass_utils import run_bass_kernel_spmd
from concourse.masks import make_identity
from concourse.vector_clock import ScopedClock

B, T, V, E, H, G = 64, 512, 32000, 512, 512, 2048  # G = 4H
NCORES = 8
BS = B // NCORES  # 8 batch rows per core
KC = E // 128     # 4 contraction chunks of 128
F32 = mybir.dt.float32
F32R = mybir.dt.float32r
I32 = mybir.dt.int32
ACT_SIG = mybir.ActivationFunctionType.Sigmoid
ACT_TANH = mybir.ActivationFunctionType.Tanh
ACT_EXP = mybir.ActivationFunctionType.Exp
ACT_COPY = mybir.ActivationFunctionType.Copy


def _patch_tile_drain():
    """walrus here rejects >2 sem-waits on one instruction; spread the
    TileContext exit-drain waits across single-wait SP NOPs."""
    if getattr(tile.TileContext, "_drain_patched", False):
        return

    MAX_WAITS = 1

    def _spread_waits(nc):
        """walrus here rejects instructions with >MAX_WAITS sem-waits; hoist
        the excess onto same-engine NOPs inserted just before the victim."""
        uid = [0]
        for f in nc.m.functions:
            for bb in f.blocks:
                out = []
                changed = False
                for inst in bb.instructions:
                    si = getattr(inst, "sync_info", None)
                    waits = list(si.on_wait) if si is not None and si.on_wait else []
                    n_upd = len(si.on_update or []) if si is not None else 0
                    budget = _patch_tile_drain.MAX_WAITS
                    if len(waits) > budget:
                        keep = waits[-budget:] if budget else []
                        excess = waits[:-budget] if budget else waits
                        for i in range(0, len(excess), _patch_tile_drain.MAX_WAITS):
                            chunk = excess[i:i + _patch_tile_drain.MAX_WAITS]
                            nop = mybir.InstNoOp(
                                name=f"waitnop_{uid[0]}", ins=[], outs=[])
                            uid[0] += 1
                            nop.engine = inst.engine
                            nop.sync_info = mybir.SyncInfo(
                                on_wait=chunk, on_update=[])
                            try:
                                nc.register_instruction(nop, overwrite=True)
                            except Exception:
                                pass
                            out.append(nop)
                        inst.sync_info = mybir.SyncInfo(
                            on_wait=keep, on_update=list(si.on_update or []))
                        changed = True
                    out.append(inst)
                if changed:
                    bb.instructions = out

    _patch_tile_drain.MAX_WAITS = MAX_WAITS

    def _drain_and_barrier(self, tick_clock, wait_clock):
        nc = self.nc
        _spread_waits(nc)
        nops = [nc.sync.nop(nofuse=True, hint=f"dw{i}") for i in range(40)]
        drain_inst = nc.sync.drain()
        wait_clock.add_sem_waits(
            drain_inst.ins, ScopedClock({None: tick_clock.global_clock})
        )
        waits = list(drain_inst.ins.sync_info.on_wait or [])
        if len(waits) > 1:
            drain_inst.ins.sync_info = mybir.SyncInfo(
                on_wait=[waits[-1]], on_update=[]
            )
            for i, w in enumerate(waits[:-1]):
                nops[i].ins.sync_info = mybir.SyncInfo(on_wait=[w], on_update=[])
        nc.all_engine_barrier()
        assert self.sems is not None
        popped = nc._tile_sem_poison_stack.pop()
        assert popped is self._sem_poison
        nc.clear_and_free_semaphores(list(self.sems.allocated().values()))
        nc.all_engine_barrier()

    tile.TileContext._drain_and_barrier = _drain_and_barrier
    tile.TileContext._drain_patched = True


def _build(t_steps=T):
    _patch_tile_drain()
    nc = bass.Bass()

    # ---- DRAM I/O (per core) ----
    idx = nc.dram_tensor("idx", [BS * t_steps], I32, kind="ExternalInput")
    emb = nc.dram_tensor("emb", [V, E], F32, kind="ExternalInput")
    wi = nc.dram_tensor("wi", [E, G], F32, kind="ExternalInput")
    wh = nc.dram_tensor("wh", [E, G], F32, kind="ExternalInput")
    bias_ig = nc.dram_tensor("bias_ig", [1, G], F32, kind="ExternalInput")  # bi+bh
    w_lstm = nc.dram_tensor("w_lstm", [H, H], F32, kind="ExternalInput")
    b_lstm = nc.dram_tensor("b_lstm", [1, H], F32, kind="ExternalInput")
    w_att = nc.dram_tensor("w_att", [2 * H, 1], F32, kind="ExternalInput")
    b_att = nc.dram_tensor("b_att", [1, 1], F32, kind="ExternalInput")

    out_sig = nc.dram_tensor("out_sig", [BS, 1], F32, kind="ExternalOutput")
    h_out = nc.dram_tensor("h_out", [BS, H], F32, kind="ExternalOutput")

    # scratch DRAM
    xp_dram = nc.dram_tensor("xp_scratch", [BS, t_steps, G], F32)
    hs_dram = nc.dram_tensor("hs_scratch", [t_steps, BS, H], F32)       # (t, b, h)
    hsT_dram = nc.dram_tensor("hsT_scratch", [KC, 128, t_steps * BS], F32R)  # (k, p, t*BS+b)

    n_tok_tiles = (BS * t_steps) // 128  # token tiles of 128 (b-major order)
    tiles_per_b = t_steps // 128

    with tile.TileContext(nc) as tc:
        # ======== constants / weights resident all kernel ========
        with contextlib.ExitStack() as stack:
            const_pool = stack.enter_context(tc.tile_pool(name="const", bufs=1))
            wh_pool = stack.enter_context(tc.tile_pool(name="whp", bufs=1))

            ident128 = const_pool.tile([128, 128], F32)
            make_identity(nc, ident128[:])
            # f32r copies of small constants
            id8_r = const_pool.tile([8, 8], F32R)
            nc.vector.tensor_copy(id8_r[:], ident128[:8, :8])
            ones8_r = const_pool.tile([1, 8], F32R)
            nc.vector.tensor_copy(ones8_r[:], ident128[:1, :1].to_broadcast([1, 8]))
            ones8_f = const_pool.tile([1, 8], F32)
            nc.vector.tensor_copy(ones8_f[:], ident128[:1, :1].to_broadcast([1, 8]))

            # Wh chunks, f32r [128, 4*G? ] -> [128, KC*G] too big? KC*G*4B = 32KB/part OK? 2048*4*4=32KB  < 224KB fine
            carry_pool = stack.enter_context(tc.tile_pool(name="carry", bufs=1))
            carry_hT = carry_pool.tile([128, KC * BS], F32R)

            wh_sb = wh_pool.tile([128, KC * G], F32R)
            for k in range(KC):
                nc.gpsimd.dma_start(wh_sb[:, k * G:(k + 1) * G], wh[k * 128:(k + 1) * 128, :])

            # ======== Phase A: gather + xp = x@Wi + bias ========
            with contextlib.ExitStack() as pa:
                wi_pool = pa.enter_context(tc.tile_pool(name="wip", bufs=1))
                ga_pool = pa.enter_context(tc.tile_pool(name="gap", bufs=3))
                xt_pool = pa.enter_context(tc.tile_pool(name="xtp", bufs=3))
                pa_psum = pa.enter_context(tc.tile_pool(name="paps", bufs=2, space="PSUM"))
                pa_psum2 = pa.enter_context(tc.tile_pool(name="paps2", bufs=1, space="PSUM"))
                xp_sb_pool = pa.enter_context(tc.tile_pool(name="xpsb", bufs=3))
                misc_pool = pa.enter_context(tc.tile_pool(name="miscp", bufs=1))

                wi_sb = wi_pool.tile([128, KC * G], F32R)
                for k in range(KC):
                    nc.gpsimd.dma_start(wi_sb[:, k * G:(k + 1) * G], wi[k * 128:(k + 1) * 128, :])
                bias_sb = misc_pool.tile([1, G], F32R)
                nc.gpsimd.dma_start(bias_sb[:], bias_ig[:])
                ones1_r = misc_pool.tile([1, 128], F32R)
                nc.vector.tensor_copy(ones1_r[:], ident128[:1, :1].to_broadcast([1, 128]))

                idx_sb = misc_pool.tile([128, n_tok_tiles], I32)
                nc.sync.dma_start(
                    idx_sb[:], idx.rearrange("(t p) -> p t", p=128)
                )

                for tt in range(n_tok_tiles):
                    x_tile = ga_pool.tile([128, E], F32, tag="x")
                    nc.gpsimd.indirect_dma_start(
                        out=x_tile[:],
                        out_offset=None,
                        in_=emb[:],
                        in_offset=bass.IndirectOffsetOnAxis(ap=idx_sb[:, tt:tt + 1], axis=0),
                    )
                    # transpose x tile: 4x [128,128] -> xT [128, 4*128]
                    xt_ps = pa_psum.tile([128, E], F32, tag="xtps")
                    for e in range(KC):
                        nc.tensor.transpose(
                            xt_ps[:, e * 128:(e + 1) * 128],
                            x_tile[:, e * 128:(e + 1) * 128],
                            ident128[:],
                        )
                    xt_sb = xt_pool.tile([128, E], F32R, tag="xt")
                    nc.scalar.activation(xt_sb[:], xt_ps[:], ACT_COPY)

                    xp_ps = pa_psum2.tile([128, G], F32, tag="xpps")
                    for n in range(4):
                        nsl = slice(n * 512, (n + 1) * 512)
                        # bias row (K=1) starts the accumulation
                        nc.tensor.matmul(
                            xp_ps[:, nsl], ones1_r[:], bias_sb[:, nsl],
                            start=True, stop=False,
                        )
                        for e in range(KC):
                            nc.tensor.matmul(
                                xp_ps[:, nsl],
                                xt_sb[:, e * 128:(e + 1) * 128],
                                wi_sb[:, e * G + n * 512: e * G + (n + 1) * 512],
                                start=False, stop=(e == KC - 1),
                            )
                    xp_sb = xp_sb_pool.tile([128, G], F32, tag="xpsb")
                    nc.scalar.activation(xp_sb[:], xp_ps[:], ACT_COPY)
                    b_i = tt // tiles_per_b
                    trow = (tt % tiles_per_b) * 128
                    nc.sync.dma_start(
                        xp_dram[b_i, trow:trow + 128, :], xp_sb[:]
                    )

            # ======== Phase B: LSTM scan ========
            with contextlib.ExitStack() as pb:
                state_pool = pb.enter_context(tc.tile_pool(name="state", bufs=1))
                xp_pool = pb.enter_context(tc.tile_pool(name="xpin", bufs=4))
                act_pool = pb.enter_context(tc.tile_pool(name="actp", bufs=2))
                ht_pool = pb.enter_context(tc.tile_pool(name="htp", bufs=2))
                g_psum = pb.enter_context(tc.tile_pool(name="gps", bufs=1, space="PSUM"))
                t_psum = pb.enter_context(tc.tile_pool(name="tps", bufs=2, space="PSUM"))

                c0 = state_pool.tile([BS, H], F32, tag="c0")
                c1 = state_pool.tile([BS, H], F32, tag="c1")
                c_state = [c0, c1]
                nc.gpsimd.memset(c_state[0][:], 0.0)
                hT_init = state_pool.tile([128, 4 * BS], F32R, tag="hT_init")
                z0 = state_pool.tile([128, 4 * BS], F32, tag="z0")
                nc.gpsimd.memset(z0[:], 0.0)
                nc.vector.tensor_copy(hT_init[:], z0[:])
                h_prev_T = hT_init

                h_sb = None
                for t in range(t_steps):
                    xp_t = xp_pool.tile([BS, G], F32R, tag="xp")
                    nc.gpsimd.dma_start(xp_t[:], xp_dram[:, t, :])

                    gate_ps = g_psum.tile([BS, G], F32, tag="g")
                    for n in range(4):
                        nsl = slice(n * 512, (n + 1) * 512)
                        nc.tensor.matmul(gate_ps[:, nsl], id8_r[:], xp_t[:, nsl],
                                         start=True, stop=False)
                        for k in range(KC):
                            nc.tensor.matmul(
                                gate_ps[:, nsl],
                                h_prev_T[:, k * BS:(k + 1) * BS],
                                wh_sb[:, k * G + n * 512: k * G + (n + 1) * 512],
                                start=False, stop=(k == KC - 1),
                            )

                    rf = act_pool.tile([BS, 2 * H], F32, tag="rf")
                    nc.scalar.activation(rf[:], gate_ps[:, 0:2 * H], ACT_SIG)
                    gt = act_pool.tile([BS, H], F32, tag="gt")
                    nc.scalar.activation(gt[:], gate_ps[:, 2 * H:3 * H], ACT_TANH)
                    ot = act_pool.tile([BS, H], F32, tag="ot")
                    nc.scalar.activation(ot[:], gate_ps[:, 3 * H:4 * H], ACT_SIG)

                    c_old = c_state[t % 2]
                    c_new = c_state[(t + 1) % 2]
                    fc = act_pool.tile([BS, H], F32, tag="fc")
                    nc.vector.tensor_mul(fc[:], rf[:, H:2 * H], c_old[:])
                    rg = act_pool.tile([BS, H], F32, tag="rg")
                    nc.vector.tensor_mul(rg[:], rf[:, 0:H], gt[:])
                    nc.vector.tensor_add(c_new[:], fc[:], rg[:])

                    tc_t = act_pool.tile([BS, H], F32, tag="tc")
                    nc.scalar.activation(tc_t[:], c_new[:], ACT_TANH)
                    h_sb = act_pool.tile([BS, H], F32, tag="h")
                    nc.vector.tensor_mul(h_sb[:], ot[:], tc_t[:])

                    # save h for attention
                    nc.sync.dma_start(hs_dram[t, :, :], h_sb[:])

                    # transpose h -> hT [128, KC*BS]
                    ht_ps = t_psum.tile([128, KC * BS], F32, tag="htps")
                    for k in range(KC):
                        nc.tensor.transpose(
                            ht_ps[:, k * BS:(k + 1) * BS],
                            h_sb[:, k * 128:(k + 1) * 128],
                            ident128[:8, :8],
                        )
                    hT = ht_pool.tile([128, KC * BS], F32R, tag="hT")
                    nc.vector.tensor_copy(hT[:], ht_ps[:])
                    for k in range(KC):
                        nc.sync.dma_start(
                            hsT_dram[k, :, t * BS:(t + 1) * BS],
                            hT[:, k * BS:(k + 1) * BS],
                        )
                    h_prev_T = hT

                nc.sync.dma_start(h_out[:], h_sb[:])
                nc.vector.tensor_copy(carry_hT[:], h_prev_T[:])

            # ======== Phase C: attention tail ========
            final_hT = carry_hT
            if True:
                with contextlib.ExitStack() as pc:
                    tail_pool = pc.enter_context(tc.tile_pool(name="tail", bufs=1))
                    big_pool = pc.enter_context(tc.tile_pool(name="big", bufs=1))
                    tl_psum = pc.enter_context(tc.tile_pool(name="tlps", bufs=1, space="PSUM"))

                    # final_hidden = h @ W_lstm + b_lstm   [BS, H]
                    wl_sb = tail_pool.tile([128, KC * H], F32R, tag="wl")
                    for k in range(KC):
                        nc.gpsimd.dma_start(wl_sb[:, k * H:(k + 1) * H], w_lstm[k * 128:(k + 1) * 128, :])
                    bl_sb = tail_pool.tile([1, H], F32R, tag="bl")
                    nc.gpsimd.dma_start(bl_sb[:], b_lstm[:])
                    fh_ps = tl_psum.tile([BS, H], F32, tag="fhps")
                    nc.tensor.matmul(fh_ps[:], ones8_r[:1, :BS], bl_sb[:], start=True, stop=False)
                    for k in range(KC):
                        nc.tensor.matmul(
                            fh_ps[:], final_hT[:, k * BS:(k + 1) * BS],
                            wl_sb[:, k * H:(k + 1) * H],
                            start=False, stop=(k == KC - 1),
                        )
                    fh_sb = tail_pool.tile([BS, H], F32, tag="fh")
                    nc.scalar.activation(fh_sb[:], fh_ps[:], ACT_COPY)

                    # fhT [128, KC*BS]
                    fhT_ps = tl_psum.tile([128, KC * BS], F32, tag="fhTps")
                    for k in range(KC):
                        nc.tensor.transpose(
                            fhT_ps[:, k * BS:(k + 1) * BS],
                            fh_sb[:, k * 128:(k + 1) * 128],
                            ident128[:8, :8],
                        )
                    fhT_sb = tail_pool.tile([128, KC * BS], F32, tag="fhT")
                    nc.vector.tensor_copy(fhT_sb[:], fhT_ps[:])

                    # score[b, t] = sum_h hs[t,b,h] fh[b,h]; via scoreT = hsT_b @ fhT_b
                    hsT_sb = big_pool.tile([128, KC * t_steps * BS], F32, tag="hsT")
                    for k in range(KC):
                        nc.gpsimd.dma_start(
                            hsT_sb[:, k * t_steps * BS:(k + 1) * t_steps * BS],
                            hsT_dram[k],
                        )
                    n_ttiles = t_steps // 128
                    scT_ps = tl_psum.tile([128, n_ttiles * BS], F32, tag="scT")
                    for tt in range(n_ttiles):
                        for b in range(BS):
                            for k in range(KC):
                                st = k * t_steps * BS + tt * 128 * BS + b
                                lhs = hsT_sb[:, st: st + 127 * BS + 1: BS]
                                nc.tensor.matmul(
                                    scT_ps[:, tt * BS + b: tt * BS + b + 1],
                                    lhs,
                                    fhT_sb[:, k * BS + b: k * BS + b + 1],
                                    start=(k == 0), stop=(k == KC - 1),
                                )
                    # transpose scoreT tiles -> score [BS, t_steps]
                    sc_ps = tl_psum.tile([BS, t_steps], F32, tag="scps")
                    scT_sb = tail_pool.tile([128, n_ttiles * BS], F32, tag="scTsb")
                    nc.vector.tensor_copy(scT_sb[:], scT_ps[:])
                    for tt in range(n_ttiles):
                        nc.tensor.transpose(
                            sc_ps[:, tt * 128:(tt + 1) * 128],
                            scT_sb[:, tt * BS:(tt + 1) * BS],
                            ident128[:],
                        )
                    score_sb = tail_pool.tile([BS, t_steps], F32, tag="score")
                    nc.vector.tensor_copy(score_sb[:], sc_ps[:])

                    # softmax over t (free dim)
                    mx = tail_pool.tile([BS, 1], F32, tag="mx")
                    nc.vector.tensor_reduce(mx[:], score_sb[:], mybir.AxisListType.X,
                                            mybir.AluOpType.max)
                    nmx = tail_pool.tile([BS, 1], F32, tag="nmx")
                    nc.vector.tensor_scalar_mul(nmx[:], mx[:], -1.0)
                    exps = tail_pool.tile([BS, t_steps], F32, tag="exps")
                    nc.scalar.activation(exps[:], score_sb[:], ACT_EXP, bias=nmx[:])
                    ssum = tail_pool.tile([BS, 1], F32, tag="ssum")
                    nc.vector.tensor_reduce(ssum[:], exps[:], mybir.AxisListType.X,
                                            mybir.AluOpType.add)
                    rsum = tail_pool.tile([BS, 1], F32, tag="rsum")
                    nc.vector.reciprocal(rsum[:], ssum[:])
                    dist = tail_pool.tile([BS, t_steps], F32, tag="dist")
                    nc.vector.tensor_scalar(
                        out=dist[:], in0=exps[:], scalar1=rsum[:], scalar2=None,
                        op0=mybir.AluOpType.mult,
                    )

                    # distT tiles [128, n_ttiles*BS]
                    dT_ps = tl_psum.tile([128, n_ttiles * BS], F32, tag="dTps")
                    for tt in range(n_ttiles):
                        nc.tensor.transpose(
                            dT_ps[:, tt * BS:(tt + 1) * BS],
                            dist[:, tt * 128:(tt + 1) * 128],
                            ident128[:8, :8],
                        )
                    dT_sb = tail_pool.tile([128, n_ttiles * BS], F32, tag="dT")
                    nc.vector.tensor_copy(dT_sb[:], dT_ps[:])

                    # att[b,h] accumulated transposed: attT [128, KC*BS]
                    hs_sb = big_pool.tile([128, n_ttiles * BS * H], F32, tag="hs")
                    for tt in range(n_ttiles):
                        nc.gpsimd.dma_start(
                            hs_sb[:, tt * BS * H:(tt + 1) * BS * H],
                            hs_dram[tt * 128:(tt + 1) * 128, :, :].rearrange(
                                "p b h -> p (b h)"),
                        )
                    attT_ps = tl_psum.tile([128, KC * BS], F32, tag="attTps")
                    for m in range(KC):      # h chunk (output partitions)
                        for b in range(BS):
                            for tt in range(n_ttiles):
                                lhs = hs_sb[:, tt * BS * H + b * H + m * 128: tt * BS * H + b * H + (m + 1) * 128]
                                nc.tensor.matmul(
                                    attT_ps[:, m * BS + b: m * BS + b + 1],
                                    lhs,
                                    dT_sb[:, tt * BS + b: tt * BS + b + 1],
                                    start=(tt == 0), stop=(tt == n_ttiles - 1),
                                )
                    attT_sb = tail_pool.tile([128, KC * BS], F32, tag="attT")
                    nc.vector.tensor_copy(attT_sb[:], attT_ps[:])

                    # out = sigmoid([fh, att] @ W_att + b_att)
                    wa_sb = tail_pool.tile([128, 8], F32, tag="wa")
                    for k in range(8):
                        nc.gpsimd.dma_start(
                            wa_sb[:, k:k + 1], w_att[k * 128:(k + 1) * 128, :]
                        )
                    ba_sb = tail_pool.tile([1, 1], F32, tag="ba")
                    nc.gpsimd.dma_start(ba_sb[:], b_att[:])
                    o_ps = tl_psum.tile([BS, 1], F32, tag="ops")
                    nc.tensor.matmul(o_ps[:], ones8_f[:1, :BS], ba_sb[:],
                                     start=True, stop=False)
                    for k in range(KC):
                        nc.tensor.matmul(o_ps[:], fhT_sb[:, k * BS:(k + 1) * BS],
                                         wa_sb[:, k:k + 1], start=False, stop=False)
                    for k in range(KC):
                        nc.tensor.matmul(o_ps[:], attT_sb[:, k * BS:(k + 1) * BS],
                                         wa_sb[:, KC + k:KC + k + 1], start=False,
                                         stop=(k == KC - 1))
                    osig = tail_pool.tile([BS, 1], F32, tag="osig")
                    nc.scalar.activation(osig[:], o_ps[:], ACT_SIG)
                    nc.sync.dma_start(out_sig[:], osig[:])

    return nc


_CACHE = {}


def _get_nc(t_steps):
    if t_steps not in _CACHE:
        _CACHE[t_steps] = _build(t_steps)
    return _CACHE[t_steps]


def kernel(input_words, emb, Wi, bi, Wh, bh, W_lstm, b_lstm, W_att, b_att,
           _t_steps=None, _trace=False):
    t_steps = _t_steps or input_words.shape[1]
    nc = _get_nc(t_steps)

    bias_ig = (bi + bh).astype(np.float32).reshape(1, G)
    emb_f = np.ascontiguousarray(emb, dtype=np.float32)
    wi_f = np.ascontiguousarray(Wi, dtype=np.float32)
    wh_f = np.ascontiguousarray(Wh, dtype=np.float32)
    wl_f = np.ascontiguousarray(W_lstm, dtype=np.float32)
    bl_f = np.ascontiguousarray(b_lstm, dtype=np.float32).reshape(1, H)
    wa_f = np.ascontiguousarray(W_att, dtype=np.float32)
    ba_f = np.ascontiguousarray(b_att, dtype=np.float32).reshape(1, 1)

    in_maps = []
    for c in range(NCORES):
        rows = input_words[c * BS:(c + 1) * BS, :t_steps]
        in_maps.append({
            "idx": np.ascontiguousarray(rows.reshape(-1).astype(np.int32)),
            "emb": emb_f, "wi": wi_f, "wh": wh_f, "bias_ig": bias_ig,
            "w_lstm": wl_f, "b_lstm": bl_f, "w_att": wa_f, "b_att": ba_f,
        })

    res = run_bass_kernel_spmd(nc, in_maps, core_ids=list(range(NCORES)),
                               trace=_trace)
    out0 = np.concatenate([r["out_sig"] for r in res.results], axis=0)
    out1 = np.concatenate([r["h_out"] for r in res.results], axis=0)
    kernel._last_result = res
    return out0, out1
